# revision 19
# baseline (speedup 1.0000x reference)
"""KD feature-level smooth-L1 loss kernel for Trainium2 (8 NeuronCores).

Math (per batch sample b over (C,H,W) = 256*64*64 = N elements):
  t_norm = (t - mean) * rsqrt(var + eps)          # LayerNorm, no affine
  d   = |t_norm - s|
  kd  = where(d <= 2, d*d/4, d - 1)               # smooth-L1, beta=2
  out = mean_b( sum_chw(kd) )

Device-side decomposition, computed directly in normalized space
(rs = 1/sqrt(var+eps)):
  x  = (s + rs*mean) - rs*t                       # = s - t_norm = -(t_norm - s)
  Q  = sum( min(x^2, BETA^2) )
  E  = sum( relu(|x| - BETA) )
  sum(kd) = 0.25*Q + E
Q and E are each ONE fused custom DVE op (affine-combine + clamp/abs +
accumulate), so the whole loss phase is 2 DVE passes per element pair.
Stats use ACT: Identity+accum (sum t) and Square+accum (sum t^2).
Sharding: pure data parallel, 4 samples per core; host combines.
"""

import re
from contextlib import ExitStack

import numpy as np

import concourse.bass as bass
import concourse.mybir as mybir
import concourse.tile as tile
from concourse import bacc, dve_ops
from concourse.bass_utils import run_bass_kernel_spmd
from concourse.dve_spec import (
    Spec, Src0, Src1, C0, C1, C2, Zero, relu, sq, maxx, minn, AluOp as DveAluOp,
)

B, C, H, W = 32, 256, 64, 64
N_CORES = 8
BPC = B // N_CORES            # samples per core
P = 128
N = C * H * W                 # 1048576 elements per sample
FD = N // P                   # 8192 free-dim per partition
NCH = 2                       # loss chunks per sample
CH = FD // NCH                # 4096
NSC = 2                       # stats chunks per sample
SC = FD // NSC                # 4096
EPS = 1e-5
BETA = 2.0
LOSS_WEIGHT = 1.0

f32 = mybir.dt.float32
bf16 = mybir.dt.bfloat16
AF = mybir.ActivationFunctionType
OP = mybir.AluOpType
AX = mybir.AxisListType


# --------------- custom fused DVE ops (Q and E passes) -----------------------
def _register_dve_op(name: str, spec: "Spec") -> "dve_ops.DveOp":
    for existing in dve_ops.OPS:
        if existing.name == name:
            return existing
    op = dve_ops.DveOp(name, spec, subdim=False, uops_sha={})
    dve_ops._SUB_OPCODE_FOR_NAME[name] = max(dve_ops._SUB_OPCODE_FOR_NAME.values()) + 1
    try:
        op.compile("v3")
    except ValueError as e:
        m = re.search(r"v3: ([0-9a-f]+)", str(e))
        if not m:
            raise
        op.uops_sha["v3"] = m.group(1)
    op.compile("v3")
    dve_ops.OPS.append(op)
    dve_ops.CUSTOM_DVE_SPECS[name] = spec
    return op


def _q_ref(in0, in1, s0, s1, imm2):
    x = (in0.astype(np.float32) + s1) - in1.astype(np.float32) * s0
    return np.minimum(x * x, imm2)


def _e_ref(in0, in1, s0, s1, imm2):
    x = (in0.astype(np.float32) + s1) - in1.astype(np.float32) * s0
    return np.maximum(np.abs(x) - imm2, 0.0)


_xq = (Src0 + C1) - Src1 * C0
Q_OP = _register_dve_op(
    "KD_SL1_Q_ANT",
    Spec(body=minn(sq(_xq), C2), accum=DveAluOp.ADD,
         reference=dve_ops._ref_body_sum(_q_ref)),
)
_xe = (Src0 + C1) - Src1 * C0
E_OP = _register_dve_op(
    "KD_SL1_E_ANT",
    Spec(body=relu(maxx(_xe, Zero - _xe) - C2), accum=DveAluOp.ADD,
         reference=dve_ops._ref_body_sum(_e_ref)),
)


def _build_kernel(ctx: ExitStack, tc: "tile.TileContext", out_ap, teacher, stu):
    nc = tc.nc

    const_pool = ctx.enter_context(tc.tile_pool(name="const", bufs=1))
    t_pool = ctx.enter_context(tc.tile_pool(name="t", bufs=2))
    s_pool = ctx.enter_context(tc.tile_pool(name="s", bufs=4))
    id_pool = ctx.enter_context(tc.tile_pool(name="iddead", bufs=2))
    sq_pool = ctx.enter_context(tc.tile_pool(name="sqdead", bufs=2))
    qdead_pool = ctx.enter_context(tc.tile_pool(name="qdead", bufs=2))
    edead_pool = ctx.enter_context(tc.tile_pool(name="edead", bufs=2))
    sums_pool = ctx.enter_context(tc.tile_pool(name="sums", bufs=3))
    tiny_pool = ctx.enter_context(tc.tile_pool(name="tiny", bufs=4))
    ps_misc_pool = ctx.enter_context(tc.tile_pool(name="ps_misc", bufs=2, space="PSUM"))

    onesf = const_pool.tile([P, 1], f32)
    nc.vector.memset(onesf[:], 1.0)
    staging = const_pool.tile([1, 16 * BPC], f32)

    for b in range(BPC):
        # ---------------- DMA: teacher halves then student chunks --------
        t_sb = t_pool.tile([P, FD], f32)
        for i in range(NSC):
            nc.scalar.dma_start(t_sb[:, i * SC : (i + 1) * SC], teacher[b, :, i * SC : (i + 1) * SC])
        s_tiles = []
        for c in range(NCH):
            s_sb = s_pool.tile([P, CH], f32)
            nc.sync.dma_start(s_sb[:], stu[b, :, c * CH : (c + 1) * CH])
            s_tiles.append(s_sb)

        # cols 0:2 sum(t) per stats chunk, 2:4 sum(t^2), 4:8 Q, 8:12 E
        sums = sums_pool.tile([P, 16], f32)

        # ---------------- stats: S_t (ACT Identity), S_tt (ACT Square) ---
        for i in range(NSC):
            sl = slice(i * SC, (i + 1) * SC)
            iddead = id_pool.tile([P, SC], bf16)
            nc.scalar.activation(iddead[:], t_sb[:, sl], AF.Identity,
                                 accum_out=sums[:, i : i + 1])
            sqdead = sq_pool.tile([P, SC], bf16)
            nc.scalar.activation(sqdead[:], t_sb[:, sl], AF.Square,
                                 accum_out=sums[:, 2 + i : 3 + i])

        ps_m = ps_misc_pool.tile([1, 16], f32)
        nc.tensor.matmul(ps_m[:, 0:4], onesf[:, :], sums[:, 0:4], start=True, stop=True)

        # ---------------- tiny scalar math -------------------------------
        # bb cols: 0=rs 1=rs*mean 2=S_t 3=S_tt 4.. scratch
        bb = tiny_pool.tile([1, 16], f32)
        st = bb[0:1, 2:3]
        nc.vector.reduce_sum(out=st, in_=ps_m[0:1, 0:2], axis=AX.X)
        stt = bb[0:1, 3:4]
        nc.vector.reduce_sum(out=stt, in_=ps_m[0:1, 2:4], axis=AX.X)
        mean = bb[0:1, 4:5]
        nc.vector.tensor_scalar(mean, st, 1.0 / N, None, op0=OP.mult)
        ve_a = bb[0:1, 5:6]
        nc.vector.tensor_scalar(ve_a, stt, 1.0 / N, EPS, op0=OP.mult, op1=OP.add)
        msq = bb[0:1, 6:7]
        nc.vector.tensor_tensor(msq, mean, mean, op=OP.mult)
        ve = bb[0:1, 7:8]
        nc.vector.tensor_tensor(ve, ve_a, msq, op=OP.subtract)
        inv_ve = bb[0:1, 8:9]
        nc.vector.reciprocal(inv_ve, ve)
        rs0 = bb[0:1, 9:10]
        nc.scalar.activation(rs0, inv_ve, AF.Sqrt)  # rs ~= 1/sqrt(ve) (table)
        # one Newton iteration: rs <- rs*(1.5 - 0.5*ve*rs^2)
        r2 = bb[0:1, 10:11]
        nc.vector.tensor_tensor(r2, rs0, rs0, op=OP.mult)
        pv = bb[0:1, 11:12]
        nc.vector.tensor_tensor(pv, r2, ve, op=OP.mult)
        hh = bb[0:1, 12:13]
        nc.vector.tensor_scalar(hh, pv, -0.5, 1.5, op0=OP.mult, op1=OP.add)
        rs = bb[0:1, 0:1]
        nc.vector.tensor_tensor(rs, rs0, hh, op=OP.mult)
        rsm = bb[0:1, 1:2]
        nc.vector.tensor_tensor(rsm, rs, mean, op=OP.mult)

        bcast = tiny_pool.tile([P, 2], f32)
        nc.gpsimd.partition_broadcast(bcast[:, 0:2], bb[0:1, 0:2])
        rs_vec = bcast[:, 0:1]
        rsm_vec = bcast[:, 1:2]

        # ---------------- loss passes: fused Q and E ---------------------
        for c in range(NCH):
            tsl = slice(c * CH, (c + 1) * CH)
            qdead = qdead_pool.tile([P, CH], bf16)
            nc.vector._custom_dve(
                Q_OP, out=qdead[:], accum_out=sums[:, 4 + c : 5 + c],
                in0=s_tiles[c][:], in1=t_sb[:, tsl], s0=rs_vec, s1=rsm_vec,
                imm2=BETA * BETA,
            )
            edead = edead_pool.tile([P, CH], bf16)
            nc.vector._custom_dve(
                E_OP, out=edead[:], accum_out=sums[:, 8 + c : 9 + c],
                in0=s_tiles[c][:], in1=t_sb[:, tsl], s0=rs_vec, s1=rsm_vec,
                imm2=BETA,
            )

        # partition-reduce the 8 loss partials in one matmul
        nc.tensor.matmul(ps_m[:, 4:12], onesf[:, :], sums[:, 4:12], start=True, stop=True)
        nc.scalar.copy(staging[0:1, 16 * b : 16 * b + 4], ps_m[0:1, 0:4])
        nc.scalar.copy(staging[0:1, 16 * b + 4 : 16 * b + 12], ps_m[0:1, 4:12])
        nc.gpsimd.tensor_copy(staging[0:1, 16 * b + 12 : 16 * b + 13], rs)
        nc.gpsimd.tensor_copy(staging[0:1, 16 * b + 13 : 16 * b + 14], rsm)
        nc.gpsimd.tensor_copy(staging[0:1, 16 * b + 14 : 16 * b + 15], ve)
        nc.gpsimd.tensor_copy(staging[0:1, 16 * b + 15 : 16 * b + 16], mean)
        nc.sync.dma_start(out_ap[:, 16 * b : 16 * b + 16], staging[:, 16 * b : 16 * b + 16])


_CACHED = {}


def _get_nc():
    if "nc" in _CACHED:
        return _CACHED["nc"]
    nc = bacc.Bacc(
        "TRN2",
        target_bir_lowering=False,
        debug=False,
        enable_asserts=False,
        num_devices=N_CORES,
    )
    teacher = nc.dram_tensor("teacher", [BPC, P, FD], f32, kind="ExternalInput").ap()
    stu = nc.dram_tensor("stu", [BPC, P, FD], f32, kind="ExternalInput").ap()
    out = nc.dram_tensor("out", [1, 16 * BPC], f32, kind="ExternalOutput").ap()
    with tile.TileContext(nc) as tc:
        with ExitStack() as ctx:
            _build_kernel(ctx, tc, out, teacher, stu)
    nc.compile()
    _CACHED["nc"] = nc
    return nc


def _combine(parts):
    """parts: list of 8 arrays [1, 16*BPC] -> scalar loss (float64 math)."""
    losses = []
    for r in parts:
        r = np.asarray(r, dtype=np.float64).reshape(BPC, 16)
        Q = r[:, 4:8].sum(axis=1)
        E = r[:, 8:12].sum(axis=1)
        losses.append(0.25 * Q + E)
    losses = np.concatenate(losses)
    return np.float32(LOSS_WEIGHT * losses.mean())


def run(inputs: dict, trace: bool = False):
    teacher = np.ascontiguousarray(np.asarray(inputs["teacher_feat"], dtype=np.float32))
    stu = np.ascontiguousarray(np.asarray(inputs["stu_feat"], dtype=np.float32))
    assert teacher.shape == (B, C, H, W) and stu.shape == (B, C, H, W)
    tch = teacher.reshape(N_CORES, BPC, P, FD)
    sch = stu.reshape(N_CORES, BPC, P, FD)
    in_maps = [
        {"teacher": np.ascontiguousarray(tch[i]), "stu": np.ascontiguousarray(sch[i])}
        for i in range(N_CORES)
    ]
    nc = _get_nc()
    res = run_bass_kernel_spmd(nc, in_maps, core_ids=list(range(N_CORES)), trace=trace)
    parts = [res.results[i]["out"] for i in range(N_CORES)]
    return _combine(parts), res


def kernel(**inputs) -> np.ndarray:
    out, _ = run(inputs, trace=False)
    return np.asarray(out, dtype=np.float32)


if __name__ == "__main__":
    rng = np.random.default_rng(0)
    ins = {
        "teacher_feat": rng.standard_normal((B, C, H, W), dtype=np.float32),
        "stu_feat": rng.standard_normal((B, C, H, W), dtype=np.float32),
    }
    print(kernel(**ins))
